# revision 54
# baseline (speedup 1.0000x reference)
"""Neural CDE encoder kernel for 8 Trainium2 NeuronCores.

Math (from the reference):
  - Natural cubic spline on unit-spaced knots; Euler times t_k = 0.05*k for
    k=0..19 all lie in interval [0,1), so only interval-0 coefficients matter:
        dX(t) = (y1 - y0) + M1 * (t^2/2 - 1/6)
    with M1 = <w, y> for a constant weight vector w over L. dX_k and z0 are
    precomputed on the host (tiny) and shipped as inputs.
  - Euler: z_{k+1} = z_k + dt * einsum('bhd,bd->bh', reshape(z W^T), dX_k)
  - Output: project grid z's with W_out, then linearly interpolate via a
    constant (L x 21) matrix.

Sharding: tensor-parallel over H (96 rows of H per core; 6144 rows of W_lin).
W^T shard SBUF-resident in bf16 (768, 6144). Per step:
  - main matmul in bf16 with 2-way col-tiling (tile_position (0,0)/(0,64)):
    kt 0-2 accumulate into psum partitions 0-63, kt 3-5 into 64-127; n-chunks
    processed in groups of 3 so each stationary load covers 3 matmuls.
  - ACT engine copies psum chunks to SBUF bf16 (pipelines behind the matmul
    burst); DVE does the dX multiply and d-reduce on bf16 double-chunks
    (1024-wide ops) against pre-broadcast bf16 dX tiles built once at
    startup; kt halves folded once per step.
  - z state kept in f32; z slice cast to bf16 and AllGathered in bf16 each
    step (half the wire bytes); gathered bf16 loads directly as the next
    stationary tiles (no post-gather cast).
  - a tiny warmup AllGather at startup absorbs the ~12us cold-collective
    cost; filler matmuls through each collective window keep the PE's HAM
    clock-gate from re-throttling the next burst.
W_out sharded over O (32 cols/core, bf16); per-step projections are emitted
after the main burst (they only feed the final output), staged to DRAM P;
the final interp matmul is split so the K=20 part runs during the last
gather and only a K=1 accumulation of P[20] trails it.
(USE_P2P: an SBUF-to-SBUF remote_dma_broadcast gather exists but is disabled
— the axon runtime wedges on SWDGE remote DMA.)
"""

import numpy as np

B, L, D, H, O = 64, 128, 64, 768, 256
NS = 20            # Euler steps
NC = 8             # cores
HLOC = H // NC     # 96
HDLOC = HLOC * D   # 6144
OLOC = O // NC     # 32
KT = H // 128      # 6 contraction tiles
NT = HDLOC // 512  # 12 moving chunks per step
NG = 4             # n-chunk groups of 3
NDC = NT // 2      # 6 double-chunks for DVE
USE_P2P = False    # hand-rolled remote-DMA gather vs collective AllGather
NFILL = 36         # HAM-warming filler matmuls per collective window


def _tile_pieces():
    """Static (tile, dst_p0, dst_p1, slot, src_p0) pieces mapping the
    8x96-row gathered slices onto 6x128-row stationary tiles."""
    pieces = []
    for t in range(KT):
        covered = 0
        while covered < 128:
            r = 128 * t + covered
            s, off = divmod(r, HLOC)
            ln = min(HLOC - off, 128 - covered)
            pieces.append((t, covered, covered + ln, s, off))
            covered += ln
    return pieces

_prog_cache = {}


def _host_constants():
    grid = (np.arange(NS + 1, dtype=np.float32) * np.float32(0.05)).astype(np.float32)
    grid[-1] = np.float32(1.0)
    dts = (grid[1:] - grid[:-1]).astype(np.float32)
    tk = grid[:-1].astype(np.float64)
    mcoef = (tk * tk / 2.0 - 1.0 / 6.0).astype(np.float32)

    # w over L such that M1 = <w, y>
    n = L - 2
    A = 4.0 * np.eye(n) + np.eye(n, k=1) + np.eye(n, k=-1)
    r0 = np.linalg.solve(A, np.eye(n)[:, 0])
    w = np.zeros(L, dtype=np.float64)
    w[0:n] += 6.0 * r0
    w[1:n + 1] += -12.0 * r0
    w[2:n + 2] += 6.0 * r0

    # Interp matrix J (L, NS+1)
    ts = np.linspace(0.0, 1.0, L, dtype=np.float32)
    j = np.clip(np.searchsorted(grid, ts, side="right") - 1, 0, NS - 1)
    wl = ((ts - grid[j]) / (grid[j + 1] - grid[j])).astype(np.float32)
    J = np.zeros((L, NS + 1), dtype=np.float32)
    J[np.arange(L), j] += 1.0 - wl
    J[np.arange(L), j + 1] += wl
    return dts, mcoef, w.astype(np.float32), J.T.copy()  # JT (21, 128)


def _build_program(dts, has_bout, ns=NS):
    import concourse.bacc as bacc
    import concourse.mybir as mybir
    import concourse.tile as tile
    from concourse.bass import ts, _add_dep_helper

    f32 = mybir.dt.float32
    bf16 = mybir.dt.bfloat16
    ADD = mybir.AluOpType.add
    MUL = mybir.AluOpType.mult
    COPY = mybir.ActivationFunctionType.Copy

    nc = bacc.Bacc("TRN2", target_bir_lowering=False, debug=False, num_devices=NC)

    # ---- I/O -------------------------------------------------------------
    wt_d = nc.dram_tensor("wt_loc", [H, HDLOC], bf16, kind="ExternalInput")
    z0t_d = nc.dram_tensor("z0t", [H, B], bf16, kind="ExternalInput")
    z0l_d = nc.dram_tensor("z0l", [HLOC, B], f32, kind="ExternalInput")
    rank_d = nc.dram_tensor("rankin", [1, 1], mybir.dt.int32, kind="ExternalInput")
    dx_d = nc.dram_tensor("dxdup", [128, NS * D], f32, kind="ExternalInput")
    wo_d = nc.dram_tensor("wo_loc", [H, OLOC], bf16, kind="ExternalInput")
    jt_d = nc.dram_tensor("jt", [NS + 1, L], f32, kind="ExternalInput")
    id_d = nc.dram_tensor("ident", [B, B], f32, kind="ExternalInput")
    if has_bout:
        bout_d = nc.dram_tensor("bout_loc", [1, OLOC], f32, kind="ExternalInput")
    out_d = nc.dram_tensor("out", [B, L, OLOC], f32, kind="ExternalOutput")

    if USE_P2P:
        rsem = nc.alloc_semaphore("rsem")
        lsem = nc.alloc_semaphore("lsem")
    zg_d = nc.dram_tensor("zgather", [H, B], bf16, kind="Internal",
                          addr_space="Shared")
    sem_patches = []  # (BassInstruction, sem, threshold) applied post-scheduling
    # tiny warmup collective buffers (acts as a startup barrier)
    wu_in_d = nc.dram_tensor("wuin", [1, 64], f32, kind="Internal")
    wu_out_d = nc.dram_tensor("wuout", [NC, 64], f32, kind="Internal",
                              addr_space="Shared")

    with tile.TileContext(nc) as tc:
        with (
            tc.tile_pool(name="pers", bufs=1) as pers,
            tc.tile_pool(name="ztpool", bufs=2) as ztp,
            tc.tile_pool(name="dram", bufs=1, space="DRAM") as dram,
        ):
            # warmup collective, independent of everything else: scheduler
            # runs it during the initial weight DMAs.
            sb_wu = pers.tile([1, 64], f32, tag="wu")
            nc.vector.memset(sb_wu[:], 0.0)
            nc.sync.dma_start(wu_in_d.ap(), sb_wu[:])
            nc.gpsimd.collective_compute(
                "AllGather", mybir.AluOpType.bypass,
                replica_groups=[list(range(NC))],
                ins=[wu_in_d.ap()], outs=[wu_out_d.ap()],
            )

            # persistent tiles
            sb_wot = pers.tile([128, KT * OLOC], bf16, tag="wot")
            for t in range(KT):
                nc.sync.dma_start(sb_wot[:, OLOC * t:OLOC * (t + 1)],
                                  wo_d[128 * t:128 * (t + 1), :])
            sb_ident = pers.tile([B, B], f32, tag="ident")
            nc.sync.dma_start(sb_ident[:], id_d[:])
            sb_ident2 = pers.tile([128, B], f32, tag="ident2")
            nc.sync.dma_start(sb_ident2[0:B, :], id_d[:])
            nc.sync.dma_start(sb_ident2[B:128, :], id_d[:])
            sb_jt = pers.tile([NS + 1, L], f32, tag="jt")
            nc.sync.dma_start(sb_jt[:], jt_d[:])
            sb_jt20 = pers.tile([1, L], f32, tag="jt20")
            nc.sync.dma_start(sb_jt20[:], jt_d[NS:NS + 1, :])
            sb_dx = pers.tile([128, NS * D], f32, tag="dx")
            nc.sync.dma_start(sb_dx[:], dx_d[:])
            sb_zsl = pers.tile([HLOC, B], f32, tag="zsl")
            nc.sync.dma_start(sb_zsl[:], z0l_d[:])
            sb_p = pers.tile([NS + 1, B * OLOC], f32, tag="P")
            if USE_P2P:
                # P2P gather state
                sb_rank = pers.tile([1, 1], mybir.dt.int32, tag="rank")
                nc.sync.dma_start(sb_rank[:], rank_d[:])
                sb_send = pers.tile([128, B], bf16, tag="send")
                nc.vector.memset(sb_send[:], 0.0)
                sb_stage = pers.tile([128, NC * B], bf16, tag="stage")
                nc.vector.memset(sb_stage[:], 0.0)
                rank_val = nc.gpsimd.value_load(sb_rank[0:1, 0:1], min_val=0,
                                                max_val=NC - 1)
            if has_bout:
                sb_bout = pers.tile([1, OLOC], f32, tag="bout")
                nc.sync.dma_start(sb_bout[:], bout_d[:])
                sb_ones = pers.tile([1, B], f32, tag="ones")
                nc.vector.memset(sb_ones[:], 1.0)

            # dX broadcast tiles for all steps, built once on ACT during the
            # startup window: [128, NS, 16, D] bf16
            sb_dxb = pers.tile([128, NS, 16, D], bf16, tag="dxb")
            for k in range(ns):
                nc.scalar.activation(
                    sb_dxb[:, k],
                    sb_dx[:, D * k:D * (k + 1)][:, None, :]
                    .to_broadcast((128, 16, D)),
                    COPY)

            # W^T shard, bf16, 6 kt-tiles side by side (128, 6*6144)
            sb_w = pers.tile([128, KT * HDLOC], bf16, tag="W")
            for t in range(KT):
                for cc in range(4):
                    nc.sync.dma_start(
                        sb_w[:, HDLOC * t + 1536 * cc:HDLOC * t + 1536 * (cc + 1)],
                        wt_d[128 * t:128 * (t + 1),
                             1536 * cc:1536 * (cc + 1)])

            # z0: gathered layout (128, 6, 64) bf16 loaded directly
            sb_zt = ztp.tile([128, KT, B], bf16, tag="zt", name="zt0")
            nc.sync.dma_start(
                sb_zt[:],
                z0t_d.ap().rearrange("(t p) b -> p t b", p=128))

            zin_d = dram.tile([HLOC, B], bf16)
            p_d = dram.tile([NS, B * OLOC], f32)
            p20_d = dram.tile([1, B * OLOC], f32)

            # ---- main loop -------------------------------------------------
            with (
                tc.tile_pool(name="work", bufs=3) as work,
                tc.tile_pool(name="tbp", bufs=3) as tbp,
                tc.tile_pool(name="t2p", bufs=2) as t2p,
                tc.tile_pool(name="upool", bufs=2) as upool,
                tc.tile_pool(name="psf", bufs=6, space="PSUM") as psf,
                tc.tile_pool(name="pst", bufs=1, space="PSUM") as pst,
                tc.tile_pool(name="psp", bufs=1, space="PSUM") as psp,
            ):
                def project(k, zt):
                    ps_p = psp.tile([128, 512], f32, tag="pp", name="ps_p")
                    if has_bout:
                        nc.tensor.matmul(ps_p[0:B, 0:OLOC], sb_ones[:], sb_bout[:],
                                         start=True, stop=False)
                    for t in range(KT):
                        nc.tensor.matmul(
                            ps_p[0:B, 0:OLOC], zt[:, t, :],
                            sb_wot[:, OLOC * t:OLOC * (t + 1)],
                            start=(t == 0 and not has_bout), stop=(t == KT - 1))
                    sb_pst = work.tile([B, OLOC], f32, tag="pstage", name="pst")
                    nc.scalar.activation(sb_pst[:], ps_p[0:B, 0:OLOC], COPY)
                    tgt = p20_d[0, :] if k == NS else p_d[k, :]
                    nc.sync.dma_start(
                        tgt.rearrange("(b o) -> b o", b=B), sb_pst[:])

                for k in range(ns):
                    sb_u = upool.tile([128, HLOC], f32, tag="U", name="u")
                    tmpb = [None] * NDC
                    for g in range(NG):
                        pss = []
                        for j in range(3):
                            ps_f = psf.tile([128, 512], f32, tag="f",
                                            name=f"ps_f{j}")
                            pss.append(ps_f)
                        for ti in range(3):
                            ta, tb = 2 * ti, 2 * ti + 1
                            for j in range(3):
                                n = 3 * g + j
                                nc.tensor.matmul(
                                    pss[j][0:64, :], sb_zt[:, ta, :],
                                    sb_w[:, HDLOC * ta + 512 * n:
                                         HDLOC * ta + 512 * (n + 1)],
                                    start=(ti == 0), stop=(ti == 2),
                                    tile_position=(0, 0))
                            for j in range(3):
                                n = 3 * g + j
                                nc.tensor.matmul(
                                    pss[j][64:128, :], sb_zt[:, tb, :],
                                    sb_w[:, HDLOC * tb + 512 * n:
                                         HDLOC * tb + 512 * (n + 1)],
                                    start=(ti == 0), stop=(ti == 2),
                                    tile_position=(0, 64))
                        for j in range(3):
                            n = 3 * g + j
                            dc, half = n // 2, n % 2
                            if tmpb[dc] is None:
                                tmpb[dc] = tbp.tile([128, 16, D], bf16,
                                                    tag="tmpb", name=f"tb{dc}")
                            # ACT: psum f32 -> sbuf bf16
                            nc.scalar.activation(
                                tmpb[dc][:, 8 * half:8 * (half + 1), :],
                                pss[j][:].rearrange("p (h d) -> p h d", d=D),
                                COPY)
                            if half == 1:
                                tmp2 = t2p.tile([128, 16, D], bf16,
                                                tag="tmp2", name="t2")
                                nc.vector.tensor_tensor(
                                    tmp2[:], tmpb[dc][:], sb_dxb[:, k], MUL)
                                nc.vector.tensor_reduce(
                                    sb_u[:, 16 * dc:16 * (dc + 1)],
                                    tmp2[:],
                                    axis=mybir.AxisListType.X, op=ADD)

                    # project z_k during the PE-idle tail (P only needed at
                    # the end)
                    project(k, sb_zt)

                    # fold kt halves: (64, 96)
                    sb_u2 = work.tile([B, HLOC], f32, tag="u2", name="u2")
                    nc.vector.tensor_copy(sb_u2[:], sb_u[64:128, :])
                    sb_uf = work.tile([B, HLOC], f32, tag="uf", name="uf")
                    nc.vector.tensor_tensor(
                        sb_uf[:], sb_u[0:64, :], sb_u2[:], ADD)

                    # transpose U -> (96, 64), update z slice, send + gather
                    ps_ut = pst.tile([128, 512], f32, tag="ut", name="ps_ut")
                    nc.tensor.transpose(ps_ut[0:HLOC, 0:B], sb_uf[:], sb_ident[:])
                    stt = nc.vector.scalar_tensor_tensor(
                        sb_zsl[:], ps_ut[0:HLOC, 0:B], float(dts[k]), sb_zsl[:],
                        op0=MUL, op1=ADD)
                    if USE_P2P:
                        # cast slice to bf16 into the send buffer; overwrite
                        # only after the previous broadcast fully sent
                        # (lsem wait patched onto a carrier nop post-sched)
                        cast = nc.vector.tensor_copy(sb_send[0:HLOC, :],
                                                     sb_zsl[:])
                        if k > 0:
                            wl = nc.vector.nop()
                            _add_dep_helper(wl.ins, stt.ins, sync=True,
                                            reason="anchor send gate in step")
                            _add_dep_helper(cast.ins, wl.ins, sync=True,
                                            reason="send buffer free gate")
                            sem_patches.append((wl, lsem, 16 * k))
                        nc.gpsimd.remote_dma_broadcast(
                            sb_stage[:, ts(rank_val, B)],
                            sb_send[:],
                            remote_sem=rsem, local_sem=lsem,
                            rdests=[(0, j) for j in range(NC)],
                        )
                        trig = nc.gpsimd.trigger_dma(count=None)
                        # carrier nop on the SP queue: gets the remote-
                        # arrival wait patched in post-scheduling
                        wr = nc.sync.nop()
                        _add_dep_helper(wr.ins, trig.ins, sync=True,
                                        reason="after p2p trigger")
                        sem_patches.append((wr, rsem, 16 * (k + 1)))
                        sb_zt = ztp.tile([128, KT, B], bf16, tag="zt",
                                         name="zt")
                        for (t, d0, d1, s, p0) in _tile_pieces():
                            mv = nc.sync.dma_start(
                                sb_zt[d0:d1, t, :],
                                sb_stage[p0:p0 + (d1 - d0),
                                         s * B:(s + 1) * B])
                            _add_dep_helper(mv.ins, wr.ins, sync=True,
                                            reason="p2p data arrival gate")
                    else:
                        sb_zslb = work.tile([HLOC, B], bf16, tag="zslb",
                                            name="zslb")
                        nc.vector.tensor_copy(sb_zslb[:], sb_zsl[:])
                        nc.sync.dma_start(zin_d[:], sb_zslb[:])
                        nc.gpsimd.collective_compute(
                            "AllGather", mybir.AluOpType.bypass,
                            replica_groups=[list(range(NC))],
                            ins=[zin_d[:]], outs=[zg_d.ap()],
                        )
                        # keep the PE HAM-warm through the collective window
                        # so the next burst runs at the warm clock
                        if NFILL and k < ns - 1:
                            ps_fill = pst.tile([128, 512], f32, tag="ut",
                                               name="ps_fill")
                            for fi in range(NFILL):
                                nc.tensor.matmul(
                                    ps_fill[0:B, :], sb_zslb[:],
                                    sb_w[0:HLOC, 512 * fi:512 * (fi + 1)],
                                    start=True, stop=True)
                        sb_zt = ztp.tile([128, KT, B], bf16, tag="zt",
                                         name="zt")
                        zg_r = zg_d.ap().rearrange("(t p) b -> p t b", p=128)
                        # adjacent tiles pair in the matmul loop, so the
                        # first pair {0,1} unblocks after a small first DMA
                        nc.sync.dma_start(sb_zt[:, 0:2, :], zg_r[:, 0:2, :])
                        nc.sync.dma_start(sb_zt[:, 2:6, :], zg_r[:, 2:6, :])

                project(ns, sb_zt)

                # ---- final interp + output --------------------------------
                # rows 0..19 of P are ready before the last gather: run the
                # big interp matmuls early; only the K=1 accumulation of
                # P[20] waits for the final projection.
                nc.sync.dma_start(sb_p[0:NS, :], p_d[:])
                sb_p20 = work.tile([1, B * OLOC], f32, tag="p20", name="p20")
                nc.sync.dma_start(sb_p20[:], p20_d[:])
                out_lbo = out_d.ap().rearrange("b l o -> l b o")
                BCH = 512 // OLOC
                for c in range(B * OLOC // 512):
                    ps_o = psf.tile([128, 512], f32, tag="f", name="ps_o")
                    nc.tensor.matmul(ps_o[0:L, :], sb_jt[0:NS, :],
                                     sb_p[0:NS, 512 * c:512 * (c + 1)],
                                     start=True, stop=False)
                    nc.tensor.matmul(ps_o[0:L, :], sb_jt20[:],
                                     sb_p20[:, 512 * c:512 * (c + 1)],
                                     start=False, stop=True)
                    sb_o = work.tile([L, 512], f32, tag="outstage", name="sb_o")
                    nc.scalar.activation(sb_o[:], ps_o[0:L, :], COPY)
                    nc.sync.dma_start(
                        out_lbo[:, BCH * c:BCH * (c + 1), :],
                        sb_o[:].rearrange("l (b o) -> l b o", o=OLOC))

    # inject the cross-core data-arrival waits after scheduling (the tile
    # scheduler's single-core sim cannot model remote semaphore increments
    # and would deadlock on them)
    for inst, sem, val in sem_patches:
        inst.wait_op(sem, val, "sem-ge", check=False)

    nc.compile()
    return nc


def _prepare(inputs):
    import ml_dtypes

    traj = np.asarray(inputs["traj"], dtype=np.float32)
    W_lin = np.asarray(inputs["W_lin"], dtype=np.float32)
    b_lin = np.asarray(inputs["b_lin"], dtype=np.float32)
    W_out = np.asarray(inputs["W_out"], dtype=np.float32)
    b_out = np.asarray(inputs["b_out"], dtype=np.float32)
    W_z0 = np.asarray(inputs["W_z0"], dtype=np.float32)
    b_z0 = np.asarray(inputs["b_z0"], dtype=np.float32)

    dts, mcoef, wv, JT = _host_constants()
    has_blin = bool(np.any(b_lin))
    has_bout = bool(np.any(b_out))
    if has_blin:
        raise NotImplementedError("b_lin != 0 not supported in fast path")

    key = (has_blin, has_bout)
    if key not in _prog_cache:
        _prog_cache[key] = _build_program(dts, has_bout)
    nc = _prog_cache[key]

    # host-side setup math (tiny)
    m1 = np.einsum('bld,l->bd', traj, wv).astype(np.float32)       # (B, D)
    base = (traj[:, 1, :] - traj[:, 0, :]).astype(np.float32)      # (B, D)
    dx = base[:, None, :] + mcoef[None, :, None] * m1[:, None, :]  # (B, NS, D)
    dx_dup = np.concatenate([dx, dx], axis=0)                      # (128, NS, D)
    dx_dup = np.ascontiguousarray(
        dx_dup.transpose(0, 1, 2).reshape(128, NS * D)).astype(np.float32)
    z0 = (traj[:, 0, :] @ W_z0.T + b_z0).astype(np.float32)        # (B, H)
    z0t = np.ascontiguousarray(z0.T)                               # (H, B)
    z0t_bf = z0t.astype(ml_dtypes.bfloat16)

    ident = np.eye(B, dtype=np.float32)
    WT_bf = np.ascontiguousarray(W_lin.T).astype(ml_dtypes.bfloat16)
    WO_bf = np.ascontiguousarray(W_out.T).astype(ml_dtypes.bfloat16)

    in_maps = []
    for i in range(NC):
        osl = slice(OLOC * i, OLOC * (i + 1))
        m = dict(
            wt_loc=np.ascontiguousarray(
                WT_bf[:, HLOC * D * i:HLOC * D * (i + 1)]),
            z0t=z0t_bf,
            z0l=np.ascontiguousarray(z0t[HLOC * i:HLOC * (i + 1), :]),
            dxdup=dx_dup,
            wo_loc=np.ascontiguousarray(WO_bf[:, osl]),
            jt=JT,
            ident=ident,
            rankin=np.array([[i]], np.int32),
        )
        if has_bout:
            m["bout_loc"] = np.ascontiguousarray(b_out[None, osl])
        in_maps.append(m)

    return nc, in_maps


def traced_run_args(inputs):
    """Build (nc, in_maps) exactly as kernel() would — for profiling."""
    return _prepare(inputs)


def kernel(**inputs):
    from concourse.bass_utils import run_bass_kernel_spmd

    nc, in_maps = _prepare(inputs)
    res = run_bass_kernel_spmd(nc, in_maps, core_ids=list(range(NC)))
    return np.concatenate([r["out"] for r in res.results], axis=2)


# revision 56
# speedup vs baseline: 1.0217x; 1.0217x over previous
"""Neural CDE encoder kernel for 8 Trainium2 NeuronCores.

Math (from the reference):
  - Natural cubic spline on unit-spaced knots; Euler times t_k = 0.05*k for
    k=0..19 all lie in interval [0,1), so only interval-0 coefficients matter:
        dX(t) = (y1 - y0) + M1 * (t^2/2 - 1/6)
    with M1 = <w, y> for a constant weight vector w over L. dX_k and z0 are
    precomputed on the host (tiny) and shipped as inputs.
  - Euler: z_{k+1} = z_k + dt * einsum('bhd,bd->bh', reshape(z W^T), dX_k)
  - Output: project grid z's with W_out, then linearly interpolate via a
    constant (L x 21) matrix.

Sharding: tensor-parallel over H (96 rows of H per core; 6144 rows of W_lin).
W^T shard SBUF-resident in bf16 (768, 6144). Per step:
  - main matmul in bf16 with 2-way col-tiling (tile_position (0,0)/(0,64)):
    kt 0-2 accumulate into psum partitions 0-63, kt 3-5 into 64-127; n-chunks
    processed in groups of 3 so each stationary load covers 3 matmuls.
  - ACT engine copies psum chunks to SBUF bf16 (pipelines behind the matmul
    burst); DVE does the dX multiply and d-reduce on bf16 double-chunks
    (1024-wide ops) against pre-broadcast bf16 dX tiles built once at
    startup; kt halves folded once per step.
  - z state kept in f32; z slice cast to bf16 and AllGathered in bf16 each
    step (half the wire bytes); gathered bf16 loads directly as the next
    stationary tiles (no post-gather cast).
  - a tiny warmup AllGather at startup absorbs the ~12us cold-collective
    cost; filler matmuls through each collective window keep the PE's HAM
    clock-gate from re-throttling the next burst.
W_out sharded over O (32 cols/core, bf16); per-step projections are emitted
after the main burst (they only feed the final output), staged to DRAM P;
the final interp matmul is split so the K=20 part runs during the last
gather and only a K=1 accumulation of P[20] trails it.
(USE_P2P: an SBUF-to-SBUF remote_dma_broadcast gather exists but is disabled
— the axon runtime wedges on SWDGE remote DMA.)
"""

import numpy as np

B, L, D, H, O = 64, 128, 64, 768, 256
NS = 20            # Euler steps
NC = 8             # cores
HLOC = H // NC     # 96
HDLOC = HLOC * D   # 6144
OLOC = O // NC     # 32
KT = H // 128      # 6 contraction tiles
NT = HDLOC // 512  # 12 moving chunks per step
NG = 4             # n-chunk groups of 3
NDC = NT // 2      # 6 double-chunks for DVE
USE_P2P = False    # hand-rolled remote-DMA gather vs collective AllGather
NFILL = 36         # HAM-warming filler matmuls per collective window


def _tile_pieces():
    """Static (tile, dst_p0, dst_p1, slot, src_p0) pieces mapping the
    8x96-row gathered slices onto 6x128-row stationary tiles."""
    pieces = []
    for t in range(KT):
        covered = 0
        while covered < 128:
            r = 128 * t + covered
            s, off = divmod(r, HLOC)
            ln = min(HLOC - off, 128 - covered)
            pieces.append((t, covered, covered + ln, s, off))
            covered += ln
    return pieces

_prog_cache = {}


def _host_constants():
    grid = (np.arange(NS + 1, dtype=np.float32) * np.float32(0.05)).astype(np.float32)
    grid[-1] = np.float32(1.0)
    dts = (grid[1:] - grid[:-1]).astype(np.float32)
    tk = grid[:-1].astype(np.float64)
    mcoef = (tk * tk / 2.0 - 1.0 / 6.0).astype(np.float32)

    # w over L such that M1 = <w, y>
    n = L - 2
    A = 4.0 * np.eye(n) + np.eye(n, k=1) + np.eye(n, k=-1)
    r0 = np.linalg.solve(A, np.eye(n)[:, 0])
    w = np.zeros(L, dtype=np.float64)
    w[0:n] += 6.0 * r0
    w[1:n + 1] += -12.0 * r0
    w[2:n + 2] += 6.0 * r0

    # Interp matrix J (L, NS+1)
    ts = np.linspace(0.0, 1.0, L, dtype=np.float32)
    j = np.clip(np.searchsorted(grid, ts, side="right") - 1, 0, NS - 1)
    wl = ((ts - grid[j]) / (grid[j + 1] - grid[j])).astype(np.float32)
    J = np.zeros((L, NS + 1), dtype=np.float32)
    J[np.arange(L), j] += 1.0 - wl
    J[np.arange(L), j + 1] += wl
    return dts, mcoef, w.astype(np.float32), J.T.copy()  # JT (21, 128)


def _build_program(dts, has_bout, ns=NS):
    import concourse.bacc as bacc
    import concourse.mybir as mybir
    import concourse.tile as tile
    from concourse.bass import ts, _add_dep_helper

    f32 = mybir.dt.float32
    bf16 = mybir.dt.bfloat16
    ADD = mybir.AluOpType.add
    MUL = mybir.AluOpType.mult
    COPY = mybir.ActivationFunctionType.Copy

    nc = bacc.Bacc("TRN2", target_bir_lowering=False, debug=False, num_devices=NC)

    # ---- I/O -------------------------------------------------------------
    wt_d = nc.dram_tensor("wt_loc", [H, HDLOC], bf16, kind="ExternalInput")
    z0t_d = nc.dram_tensor("z0t", [H, B], bf16, kind="ExternalInput")
    z0l_d = nc.dram_tensor("z0l", [HLOC, B], f32, kind="ExternalInput")
    rank_d = nc.dram_tensor("rankin", [1, 1], mybir.dt.int32, kind="ExternalInput")
    dx_d = nc.dram_tensor("dxdup", [128, NS * D], f32, kind="ExternalInput")
    wo_d = nc.dram_tensor("wo_loc", [H, OLOC], bf16, kind="ExternalInput")
    jt_d = nc.dram_tensor("jt", [NS + 1, L], f32, kind="ExternalInput")
    id_d = nc.dram_tensor("ident", [B, B], f32, kind="ExternalInput")
    if has_bout:
        bout_d = nc.dram_tensor("bout_loc", [1, OLOC], f32, kind="ExternalInput")
    out_d = nc.dram_tensor("out", [B, L, OLOC], f32, kind="ExternalOutput")

    if USE_P2P:
        rsem = nc.alloc_semaphore("rsem")
        lsem = nc.alloc_semaphore("lsem")
    zg_d = nc.dram_tensor("zgather", [H, B], bf16, kind="Internal",
                          addr_space="Shared")
    sem_patches = []  # (BassInstruction, sem, threshold) applied post-scheduling
    # tiny warmup collective buffers (acts as a startup barrier)
    wu_in_d = nc.dram_tensor("wuin", [1, 64], f32, kind="Internal")
    wu_out_d = nc.dram_tensor("wuout", [NC, 64], f32, kind="Internal",
                              addr_space="Shared")

    with tile.TileContext(nc) as tc:
        with (
            tc.tile_pool(name="pers", bufs=1) as pers,
            tc.tile_pool(name="ztpool", bufs=2) as ztp,
            tc.tile_pool(name="dram", bufs=1, space="DRAM") as dram,
        ):
            # warmup collective, independent of everything else: scheduler
            # runs it during the initial weight DMAs.
            sb_wu = pers.tile([1, 64], f32, tag="wu")
            nc.vector.memset(sb_wu[:], 0.0)
            nc.sync.dma_start(wu_in_d.ap(), sb_wu[:])
            nc.gpsimd.collective_compute(
                "AllGather", mybir.AluOpType.bypass,
                replica_groups=[list(range(NC))],
                ins=[wu_in_d.ap()], outs=[wu_out_d.ap()],
            )

            # persistent tiles
            sb_wot = pers.tile([128, KT * OLOC], bf16, tag="wot")
            for t in range(KT):
                nc.sync.dma_start(sb_wot[:, OLOC * t:OLOC * (t + 1)],
                                  wo_d[128 * t:128 * (t + 1), :])
            sb_ident = pers.tile([B, B], f32, tag="ident")
            nc.sync.dma_start(sb_ident[:], id_d[:])
            sb_ident2 = pers.tile([128, B], f32, tag="ident2")
            nc.sync.dma_start(sb_ident2[0:B, :], id_d[:])
            nc.sync.dma_start(sb_ident2[B:128, :], id_d[:])
            sb_jt = pers.tile([NS + 1, L], f32, tag="jt")
            nc.sync.dma_start(sb_jt[:], jt_d[:])
            sb_jt20 = pers.tile([1, L], f32, tag="jt20")
            nc.sync.dma_start(sb_jt20[:], jt_d[NS:NS + 1, :])
            sb_dx = pers.tile([128, NS * D], f32, tag="dx")
            nc.sync.dma_start(sb_dx[:], dx_d[:])
            sb_zsl = pers.tile([HLOC, B], f32, tag="zsl")
            nc.sync.dma_start(sb_zsl[:], z0l_d[:])
            sb_p = pers.tile([NS + 1, B * OLOC], f32, tag="P")
            if USE_P2P:
                # P2P gather state
                sb_rank = pers.tile([1, 1], mybir.dt.int32, tag="rank")
                nc.sync.dma_start(sb_rank[:], rank_d[:])
                sb_send = pers.tile([128, B], bf16, tag="send")
                nc.vector.memset(sb_send[:], 0.0)
                sb_stage = pers.tile([128, NC * B], bf16, tag="stage")
                nc.vector.memset(sb_stage[:], 0.0)
                rank_val = nc.gpsimd.value_load(sb_rank[0:1, 0:1], min_val=0,
                                                max_val=NC - 1)
            if has_bout:
                sb_bout = pers.tile([1, OLOC], f32, tag="bout")
                nc.sync.dma_start(sb_bout[:], bout_d[:])
                sb_ones = pers.tile([1, B], f32, tag="ones")
                nc.vector.memset(sb_ones[:], 1.0)

            # dX broadcast tiles for all steps, built once on ACT during the
            # startup window: [128, NS, 16, D] bf16
            sb_dxb = pers.tile([128, NS, 16, D], bf16, tag="dxb")
            for k in range(ns):
                nc.scalar.activation(
                    sb_dxb[:, k],
                    sb_dx[:, D * k:D * (k + 1)][:, None, :]
                    .to_broadcast((128, 16, D)),
                    COPY)

            # W^T shard, bf16, 6 kt-tiles side by side (128, 6*6144)
            sb_w = pers.tile([128, KT * HDLOC], bf16, tag="W")
            for t in range(KT):
                for cc in range(4):
                    nc.sync.dma_start(
                        sb_w[:, HDLOC * t + 1536 * cc:HDLOC * t + 1536 * (cc + 1)],
                        wt_d[128 * t:128 * (t + 1),
                             1536 * cc:1536 * (cc + 1)])

            # z0: gathered layout (128, 6, 64) bf16 loaded directly
            sb_zt = ztp.tile([128, KT, B], bf16, tag="zt", name="zt0")
            nc.sync.dma_start(
                sb_zt[:],
                z0t_d.ap().rearrange("(t p) b -> p t b", p=128))

            zin_d = dram.tile([HLOC, B], bf16)
            p_d = dram.tile([NS, B * OLOC], f32)
            p20_d = dram.tile([1, B * OLOC], f32)

            # ---- main loop -------------------------------------------------
            with (
                tc.tile_pool(name="work", bufs=3) as work,
                tc.tile_pool(name="tbp", bufs=3) as tbp,
                tc.tile_pool(name="t2p", bufs=2) as t2p,
                tc.tile_pool(name="upool", bufs=2) as upool,
                tc.tile_pool(name="psf", bufs=6, space="PSUM") as psf,
                tc.tile_pool(name="pst", bufs=1, space="PSUM") as pst,
                tc.tile_pool(name="psp", bufs=1, space="PSUM") as psp,
            ):
                def project(k, zt):
                    ps_p = psp.tile([128, 512], f32, tag="pp", name="ps_p")
                    if has_bout:
                        nc.tensor.matmul(ps_p[0:B, 0:OLOC], sb_ones[:], sb_bout[:],
                                         start=True, stop=False)
                    for t in range(KT):
                        nc.tensor.matmul(
                            ps_p[0:B, 0:OLOC], zt[:, t, :],
                            sb_wot[:, OLOC * t:OLOC * (t + 1)],
                            start=(t == 0 and not has_bout), stop=(t == KT - 1))
                    sb_pst = work.tile([B, OLOC], f32, tag="pstage", name="pst")
                    nc.scalar.activation(sb_pst[:], ps_p[0:B, 0:OLOC], COPY)
                    tgt = p20_d[0, :] if k == NS else p_d[k, :]
                    nc.sync.dma_start(
                        tgt.rearrange("(b o) -> b o", b=B), sb_pst[:])

                for k in range(ns):
                    sb_u = upool.tile([128, HLOC], f32, tag="U", name="u")
                    tmpb = [None] * NDC
                    for g in range(NG):
                        pss = []
                        for j in range(3):
                            ps_f = psf.tile([128, 512], f32, tag="f",
                                            name=f"ps_f{j}")
                            pss.append(ps_f)
                        for ti in range(3):
                            ta, tb = ti, ti + 3
                            for j in range(3):
                                n = 3 * g + j
                                nc.tensor.matmul(
                                    pss[j][0:64, :], sb_zt[:, ta, :],
                                    sb_w[:, HDLOC * ta + 512 * n:
                                         HDLOC * ta + 512 * (n + 1)],
                                    start=(ti == 0), stop=(ti == 2),
                                    tile_position=(0, 0))
                            for j in range(3):
                                n = 3 * g + j
                                nc.tensor.matmul(
                                    pss[j][64:128, :], sb_zt[:, tb, :],
                                    sb_w[:, HDLOC * tb + 512 * n:
                                         HDLOC * tb + 512 * (n + 1)],
                                    start=(ti == 0), stop=(ti == 2),
                                    tile_position=(0, 64))
                        for j in range(3):
                            n = 3 * g + j
                            dc, half = n // 2, n % 2
                            if tmpb[dc] is None:
                                tmpb[dc] = tbp.tile([128, 16, D], bf16,
                                                    tag="tmpb", name=f"tb{dc}")
                            # ACT: psum f32 -> sbuf bf16
                            nc.scalar.activation(
                                tmpb[dc][:, 8 * half:8 * (half + 1), :],
                                pss[j][:].rearrange("p (h d) -> p h d", d=D),
                                COPY)
                            if half == 1:
                                tmp2 = t2p.tile([128, 16, D], bf16,
                                                tag="tmp2", name="t2")
                                nc.vector.tensor_tensor(
                                    tmp2[:], tmpb[dc][:], sb_dxb[:, k], MUL)
                                nc.vector.tensor_reduce(
                                    sb_u[:, 16 * dc:16 * (dc + 1)],
                                    tmp2[:],
                                    axis=mybir.AxisListType.X, op=ADD)

                    # project z_k during the PE-idle tail (P only needed at
                    # the end)
                    project(k, sb_zt)

                    # fold kt halves: (64, 96)
                    sb_u2 = work.tile([B, HLOC], f32, tag="u2", name="u2")
                    nc.vector.tensor_copy(sb_u2[:], sb_u[64:128, :])
                    sb_uf = work.tile([B, HLOC], f32, tag="uf", name="uf")
                    nc.vector.tensor_tensor(
                        sb_uf[:], sb_u[0:64, :], sb_u2[:], ADD)

                    # transpose U -> (96, 64), update z slice, send + gather
                    ps_ut = pst.tile([128, 512], f32, tag="ut", name="ps_ut")
                    nc.tensor.transpose(ps_ut[0:HLOC, 0:B], sb_uf[:], sb_ident[:])
                    stt = nc.vector.scalar_tensor_tensor(
                        sb_zsl[:], ps_ut[0:HLOC, 0:B], float(dts[k]), sb_zsl[:],
                        op0=MUL, op1=ADD)
                    if USE_P2P:
                        # cast slice to bf16 into the send buffer; overwrite
                        # only after the previous broadcast fully sent
                        # (lsem wait patched onto a carrier nop post-sched)
                        cast = nc.vector.tensor_copy(sb_send[0:HLOC, :],
                                                     sb_zsl[:])
                        if k > 0:
                            wl = nc.vector.nop()
                            _add_dep_helper(wl.ins, stt.ins, sync=True,
                                            reason="anchor send gate in step")
                            _add_dep_helper(cast.ins, wl.ins, sync=True,
                                            reason="send buffer free gate")
                            sem_patches.append((wl, lsem, 16 * k))
                        nc.gpsimd.remote_dma_broadcast(
                            sb_stage[:, ts(rank_val, B)],
                            sb_send[:],
                            remote_sem=rsem, local_sem=lsem,
                            rdests=[(0, j) for j in range(NC)],
                        )
                        trig = nc.gpsimd.trigger_dma(count=None)
                        # carrier nop on the SP queue: gets the remote-
                        # arrival wait patched in post-scheduling
                        wr = nc.sync.nop()
                        _add_dep_helper(wr.ins, trig.ins, sync=True,
                                        reason="after p2p trigger")
                        sem_patches.append((wr, rsem, 16 * (k + 1)))
                        sb_zt = ztp.tile([128, KT, B], bf16, tag="zt",
                                         name="zt")
                        for (t, d0, d1, s, p0) in _tile_pieces():
                            mv = nc.sync.dma_start(
                                sb_zt[d0:d1, t, :],
                                sb_stage[p0:p0 + (d1 - d0),
                                         s * B:(s + 1) * B])
                            _add_dep_helper(mv.ins, wr.ins, sync=True,
                                            reason="p2p data arrival gate")
                    else:
                        sb_zslb = work.tile([HLOC, B], bf16, tag="zslb",
                                            name="zslb")
                        nc.vector.tensor_copy(sb_zslb[:], sb_zsl[:])
                        nc.sync.dma_start(zin_d[:], sb_zslb[:])
                        nc.gpsimd.collective_compute(
                            "AllGather", mybir.AluOpType.bypass,
                            replica_groups=[list(range(NC))],
                            ins=[zin_d[:]], outs=[zg_d.ap()],
                        )
                        # keep the PE HAM-warm through the collective window
                        # so the next burst runs at the warm clock
                        if NFILL and k < ns - 1:
                            ps_fill = pst.tile([128, 512], f32, tag="ut",
                                               name="ps_fill")
                            for fi in range(NFILL):
                                nc.tensor.matmul(
                                    ps_fill[0:B, :], sb_zslb[:],
                                    sb_w[0:HLOC, 512 * fi:512 * (fi + 1)],
                                    start=True, stop=True)
                        sb_zt = ztp.tile([128, KT, B], bf16, tag="zt",
                                         name="zt")
                        nc.sync.dma_start(
                            sb_zt[:],
                            zg_d.ap().rearrange("(t p) b -> p t b", p=128))

                project(ns, sb_zt)

                # ---- final interp + output --------------------------------
                # rows 0..19 of P are ready before the last gather: run the
                # big interp matmuls early; only the K=1 accumulation of
                # P[20] waits for the final projection.
                nc.sync.dma_start(sb_p[0:NS, :], p_d[:])
                sb_p20 = work.tile([1, B * OLOC], f32, tag="p20", name="p20")
                nc.sync.dma_start(sb_p20[:], p20_d[:])
                out_lbo = out_d.ap().rearrange("b l o -> l b o")
                BCH = 512 // OLOC
                for c in range(B * OLOC // 512):
                    ps_o = psf.tile([128, 512], f32, tag="f", name="ps_o")
                    nc.tensor.matmul(ps_o[0:L, :], sb_jt[0:NS, :],
                                     sb_p[0:NS, 512 * c:512 * (c + 1)],
                                     start=True, stop=False)
                    nc.tensor.matmul(ps_o[0:L, :], sb_jt20[:],
                                     sb_p20[:, 512 * c:512 * (c + 1)],
                                     start=False, stop=True)
                    sb_o = work.tile([L, 512], f32, tag="outstage", name="sb_o")
                    nc.scalar.activation(sb_o[:], ps_o[0:L, :], COPY)
                    nc.sync.dma_start(
                        out_lbo[:, BCH * c:BCH * (c + 1), :],
                        sb_o[:].rearrange("l (b o) -> l b o", o=OLOC))

    # inject the cross-core data-arrival waits after scheduling (the tile
    # scheduler's single-core sim cannot model remote semaphore increments
    # and would deadlock on them)
    for inst, sem, val in sem_patches:
        inst.wait_op(sem, val, "sem-ge", check=False)

    nc.compile()
    return nc


def _prepare(inputs):
    import ml_dtypes

    traj = np.asarray(inputs["traj"], dtype=np.float32)
    W_lin = np.asarray(inputs["W_lin"], dtype=np.float32)
    b_lin = np.asarray(inputs["b_lin"], dtype=np.float32)
    W_out = np.asarray(inputs["W_out"], dtype=np.float32)
    b_out = np.asarray(inputs["b_out"], dtype=np.float32)
    W_z0 = np.asarray(inputs["W_z0"], dtype=np.float32)
    b_z0 = np.asarray(inputs["b_z0"], dtype=np.float32)

    dts, mcoef, wv, JT = _host_constants()
    has_blin = bool(np.any(b_lin))
    has_bout = bool(np.any(b_out))
    if has_blin:
        raise NotImplementedError("b_lin != 0 not supported in fast path")

    key = (has_blin, has_bout)
    if key not in _prog_cache:
        _prog_cache[key] = _build_program(dts, has_bout)
    nc = _prog_cache[key]

    # host-side setup math (tiny)
    m1 = np.einsum('bld,l->bd', traj, wv).astype(np.float32)       # (B, D)
    base = (traj[:, 1, :] - traj[:, 0, :]).astype(np.float32)      # (B, D)
    dx = base[:, None, :] + mcoef[None, :, None] * m1[:, None, :]  # (B, NS, D)
    dx_dup = np.concatenate([dx, dx], axis=0)                      # (128, NS, D)
    dx_dup = np.ascontiguousarray(
        dx_dup.transpose(0, 1, 2).reshape(128, NS * D)).astype(np.float32)
    z0 = (traj[:, 0, :] @ W_z0.T + b_z0).astype(np.float32)        # (B, H)
    z0t = np.ascontiguousarray(z0.T)                               # (H, B)
    z0t_bf = z0t.astype(ml_dtypes.bfloat16)

    ident = np.eye(B, dtype=np.float32)
    WT_bf = np.ascontiguousarray(W_lin.T).astype(ml_dtypes.bfloat16)
    WO_bf = np.ascontiguousarray(W_out.T).astype(ml_dtypes.bfloat16)

    in_maps = []
    for i in range(NC):
        osl = slice(OLOC * i, OLOC * (i + 1))
        m = dict(
            wt_loc=np.ascontiguousarray(
                WT_bf[:, HLOC * D * i:HLOC * D * (i + 1)]),
            z0t=z0t_bf,
            z0l=np.ascontiguousarray(z0t[HLOC * i:HLOC * (i + 1), :]),
            dxdup=dx_dup,
            wo_loc=np.ascontiguousarray(WO_bf[:, osl]),
            jt=JT,
            ident=ident,
            rankin=np.array([[i]], np.int32),
        )
        if has_bout:
            m["bout_loc"] = np.ascontiguousarray(b_out[None, osl])
        in_maps.append(m)

    return nc, in_maps


def traced_run_args(inputs):
    """Build (nc, in_maps) exactly as kernel() would — for profiling."""
    return _prepare(inputs)


def kernel(**inputs):
    from concourse.bass_utils import run_bass_kernel_spmd

    nc, in_maps = _prepare(inputs)
    res = run_bass_kernel_spmd(nc, in_maps, core_ids=list(range(NC)))
    return np.concatenate([r["out"] for r in res.results], axis=2)


# revision 59
# speedup vs baseline: 1.0345x; 1.0125x over previous
"""Neural CDE encoder kernel for 8 Trainium2 NeuronCores.

Math (from the reference):
  - Natural cubic spline on unit-spaced knots; Euler times t_k = 0.05*k for
    k=0..19 all lie in interval [0,1), so only interval-0 coefficients matter:
        dX(t) = (y1 - y0) + M1 * (t^2/2 - 1/6)
    with M1 = <w, y> for a constant weight vector w over L. dX_k and z0 are
    precomputed on the host (tiny) and shipped as inputs.
  - Euler: z_{k+1} = z_k + dt * einsum('bhd,bd->bh', reshape(z W^T), dX_k)
  - Output: project grid z's with W_out, then linearly interpolate via a
    constant (L x 21) matrix.

Sharding: tensor-parallel over H (96 rows of H per core; 6144 rows of W_lin).
W^T shard SBUF-resident in bf16 (768, 6144). Per step:
  - main matmul in bf16 with 2-way col-tiling (tile_position (0,0)/(0,64)):
    kt 0-2 accumulate into psum partitions 0-63, kt 3-5 into 64-127; n-chunks
    processed in groups of 3 so each stationary load covers 3 matmuls.
  - ACT engine copies psum chunks to SBUF bf16 (pipelines behind the matmul
    burst); DVE does the dX multiply and d-reduce on bf16 double-chunks
    (1024-wide ops) against pre-broadcast bf16 dX tiles built once at
    startup; kt halves folded once per step.
  - z state kept in f32; z slice cast to bf16 and AllGathered in bf16 each
    step (half the wire bytes); gathered bf16 loads directly as the next
    stationary tiles (no post-gather cast).
  - a tiny warmup AllGather at startup absorbs the ~12us cold-collective
    cost; filler matmuls through each collective window keep the PE's HAM
    clock-gate from re-throttling the next burst.
W_out sharded over O (32 cols/core, bf16); per-step projections are emitted
after the main burst (they only feed the final output), staged to DRAM P;
the final interp matmul is split so the K=20 part runs during the last
gather and only a K=1 accumulation of P[20] trails it.
(USE_P2P: an SBUF-to-SBUF remote_dma_broadcast gather exists but is disabled
— the axon runtime wedges on SWDGE remote DMA.)
"""

import numpy as np

B, L, D, H, O = 64, 128, 64, 768, 256
NS = 20            # Euler steps
NC = 8             # cores
HLOC = H // NC     # 96
HDLOC = HLOC * D   # 6144
OLOC = O // NC     # 32
KT = H // 128      # 6 contraction tiles
NT = HDLOC // 512  # 12 moving chunks per step
NG = 4             # n-chunk groups of 3
NDC = NT // 2      # 6 double-chunks for DVE
USE_P2P = False    # hand-rolled remote-DMA gather vs collective AllGather
NFILL = 36         # HAM-warming filler matmuls per collective window


def _tile_pieces():
    """Static (tile, dst_p0, dst_p1, slot, src_p0) pieces mapping the
    8x96-row gathered slices onto 6x128-row stationary tiles."""
    pieces = []
    for t in range(KT):
        covered = 0
        while covered < 128:
            r = 128 * t + covered
            s, off = divmod(r, HLOC)
            ln = min(HLOC - off, 128 - covered)
            pieces.append((t, covered, covered + ln, s, off))
            covered += ln
    return pieces

_prog_cache = {}


def _host_constants():
    grid = (np.arange(NS + 1, dtype=np.float32) * np.float32(0.05)).astype(np.float32)
    grid[-1] = np.float32(1.0)
    dts = (grid[1:] - grid[:-1]).astype(np.float32)
    tk = grid[:-1].astype(np.float64)
    mcoef = (tk * tk / 2.0 - 1.0 / 6.0).astype(np.float32)

    # w over L such that M1 = <w, y>
    n = L - 2
    A = 4.0 * np.eye(n) + np.eye(n, k=1) + np.eye(n, k=-1)
    r0 = np.linalg.solve(A, np.eye(n)[:, 0])
    w = np.zeros(L, dtype=np.float64)
    w[0:n] += 6.0 * r0
    w[1:n + 1] += -12.0 * r0
    w[2:n + 2] += 6.0 * r0

    # Interp matrix J (L, NS+1)
    ts = np.linspace(0.0, 1.0, L, dtype=np.float32)
    j = np.clip(np.searchsorted(grid, ts, side="right") - 1, 0, NS - 1)
    wl = ((ts - grid[j]) / (grid[j + 1] - grid[j])).astype(np.float32)
    J = np.zeros((L, NS + 1), dtype=np.float32)
    J[np.arange(L), j] += 1.0 - wl
    J[np.arange(L), j + 1] += wl
    return dts, mcoef, w.astype(np.float32), J.T.copy()  # JT (21, 128)


def _build_program(dts, has_bout, ns=NS):
    import concourse.bacc as bacc
    import concourse.mybir as mybir
    import concourse.tile as tile
    from concourse.bass import ts, _add_dep_helper

    f32 = mybir.dt.float32
    bf16 = mybir.dt.bfloat16
    ADD = mybir.AluOpType.add
    MUL = mybir.AluOpType.mult
    COPY = mybir.ActivationFunctionType.Copy

    nc = bacc.Bacc("TRN2", target_bir_lowering=False, debug=False, num_devices=NC)

    # ---- I/O -------------------------------------------------------------
    wt_d = nc.dram_tensor("wt_loc", [H, HDLOC], bf16, kind="ExternalInput")
    z0t_d = nc.dram_tensor("z0t", [H, B], bf16, kind="ExternalInput")
    z0l_d = nc.dram_tensor("z0l", [HLOC, B], f32, kind="ExternalInput")
    rank_d = nc.dram_tensor("rankin", [1, 1], mybir.dt.int32, kind="ExternalInput")
    dx_d = nc.dram_tensor("dxdup", [128, NS * D], f32, kind="ExternalInput")
    wo_d = nc.dram_tensor("wo_loc", [H, OLOC], bf16, kind="ExternalInput")
    jt_d = nc.dram_tensor("jt", [NS + 1, L], f32, kind="ExternalInput")
    id_d = nc.dram_tensor("ident", [B, B], f32, kind="ExternalInput")
    if has_bout:
        bout_d = nc.dram_tensor("bout_loc", [1, OLOC], f32, kind="ExternalInput")
    out_d = nc.dram_tensor("out", [B, L, OLOC], f32, kind="ExternalOutput")

    if USE_P2P:
        rsem = nc.alloc_semaphore("rsem")
        lsem = nc.alloc_semaphore("lsem")
    zg_d = nc.dram_tensor("zgather", [H, B], bf16, kind="Internal",
                          addr_space="Shared")
    sem_patches = []  # (BassInstruction, sem, threshold) applied post-scheduling
    # tiny warmup collective buffers (acts as a startup barrier)
    wu_in_d = nc.dram_tensor("wuin", [1, 64], f32, kind="Internal")
    wu_out_d = nc.dram_tensor("wuout", [NC, 64], f32, kind="Internal",
                              addr_space="Shared")

    with tile.TileContext(nc) as tc:
        with (
            tc.tile_pool(name="pers", bufs=1) as pers,
            tc.tile_pool(name="ztpool", bufs=2) as ztp,
            tc.tile_pool(name="dram", bufs=1, space="DRAM") as dram,
        ):
            # warmup collective, independent of everything else: scheduler
            # runs it during the initial weight DMAs.
            sb_wu = pers.tile([1, 64], f32, tag="wu")
            nc.vector.memset(sb_wu[:], 0.0)
            nc.sync.dma_start(wu_in_d.ap(), sb_wu[:])
            nc.gpsimd.collective_compute(
                "AllGather", mybir.AluOpType.bypass,
                replica_groups=[list(range(NC))],
                ins=[wu_in_d.ap()], outs=[wu_out_d.ap()],
            )

            # z0 stationary first: step-0's burst only needs this + W, and
            # can then fully overlap the startup barrier
            sb_zt = ztp.tile([128, KT, B], bf16, tag="zt", name="zt0")
            nc.sync.dma_start(
                sb_zt[:],
                z0t_d.ap().rearrange("(t p) b -> p t b", p=128))

            # persistent tiles
            sb_wot = pers.tile([128, KT * OLOC], bf16, tag="wot")
            for t in range(KT):
                nc.sync.dma_start(sb_wot[:, OLOC * t:OLOC * (t + 1)],
                                  wo_d[128 * t:128 * (t + 1), :])
            sb_ident = pers.tile([B, B], f32, tag="ident")
            nc.sync.dma_start(sb_ident[:], id_d[:])
            sb_ident2 = pers.tile([128, B], f32, tag="ident2")
            nc.sync.dma_start(sb_ident2[0:B, :], id_d[:])
            nc.sync.dma_start(sb_ident2[B:128, :], id_d[:])
            sb_jt = pers.tile([NS + 1, L], f32, tag="jt")
            nc.sync.dma_start(sb_jt[:], jt_d[:])
            sb_jt20 = pers.tile([1, L], f32, tag="jt20")
            nc.sync.dma_start(sb_jt20[:], jt_d[NS:NS + 1, :])
            sb_dx = pers.tile([128, NS * D], f32, tag="dx")
            nc.sync.dma_start(sb_dx[:], dx_d[:])
            sb_zsl = pers.tile([HLOC, B], f32, tag="zsl")
            nc.sync.dma_start(sb_zsl[:], z0l_d[:])
            sb_p = pers.tile([NS + 1, B * OLOC], f32, tag="P")
            if USE_P2P:
                # P2P gather state
                sb_rank = pers.tile([1, 1], mybir.dt.int32, tag="rank")
                nc.sync.dma_start(sb_rank[:], rank_d[:])
                sb_send = pers.tile([128, B], bf16, tag="send")
                nc.vector.memset(sb_send[:], 0.0)
                sb_stage = pers.tile([128, NC * B], bf16, tag="stage")
                nc.vector.memset(sb_stage[:], 0.0)
                rank_val = nc.gpsimd.value_load(sb_rank[0:1, 0:1], min_val=0,
                                                max_val=NC - 1)
            if has_bout:
                sb_bout = pers.tile([1, OLOC], f32, tag="bout")
                nc.sync.dma_start(sb_bout[:], bout_d[:])
                sb_ones = pers.tile([1, B], f32, tag="ones")
                nc.vector.memset(sb_ones[:], 1.0)

            # dX broadcast tiles for all steps, built once on ACT during the
            # startup window: [128, NS, 16, D] bf16
            sb_dxb = pers.tile([128, NS, 16, D], bf16, tag="dxb")
            for k in range(ns):
                nc.scalar.activation(
                    sb_dxb[:, k],
                    sb_dx[:, D * k:D * (k + 1)][:, None, :]
                    .to_broadcast((128, 16, D)),
                    COPY)

            # W^T shard, bf16, 6 kt-tiles side by side (128, 6*6144)
            sb_w = pers.tile([128, KT * HDLOC], bf16, tag="W")
            for t in range(KT):
                for cc in range(4):
                    nc.sync.dma_start(
                        sb_w[:, HDLOC * t + 1536 * cc:HDLOC * t + 1536 * (cc + 1)],
                        wt_d[128 * t:128 * (t + 1),
                             1536 * cc:1536 * (cc + 1)])

            zin_d = dram.tile([HLOC, B], bf16)
            p_d = dram.tile([NS, B * OLOC], f32)
            p20_d = dram.tile([1, B * OLOC], f32)

            # ---- main loop -------------------------------------------------
            with (
                tc.tile_pool(name="work", bufs=3) as work,
                tc.tile_pool(name="tbp", bufs=3) as tbp,
                tc.tile_pool(name="t2p", bufs=2) as t2p,
                tc.tile_pool(name="upool", bufs=2) as upool,
                tc.tile_pool(name="psf", bufs=6, space="PSUM") as psf,
                tc.tile_pool(name="pst", bufs=1, space="PSUM") as pst,
                tc.tile_pool(name="psp", bufs=1, space="PSUM") as psp,
            ):
                def project(k, zt):
                    ps_p = psp.tile([128, 512], f32, tag="pp", name="ps_p")
                    if has_bout:
                        nc.tensor.matmul(ps_p[0:B, 0:OLOC], sb_ones[:], sb_bout[:],
                                         start=True, stop=False)
                    for t in range(KT):
                        nc.tensor.matmul(
                            ps_p[0:B, 0:OLOC], zt[:, t, :],
                            sb_wot[:, OLOC * t:OLOC * (t + 1)],
                            start=(t == 0 and not has_bout), stop=(t == KT - 1))
                    sb_pst = work.tile([B, OLOC], f32, tag="pstage", name="pst")
                    nc.scalar.activation(sb_pst[:], ps_p[0:B, 0:OLOC], COPY)
                    tgt = p20_d[0, :] if k == NS else p_d[k, :]
                    nc.sync.dma_start(
                        tgt.rearrange("(b o) -> b o", b=B), sb_pst[:])

                for k in range(ns):
                    sb_u = upool.tile([128, HLOC], f32, tag="U", name="u")
                    tmpb = [None] * NDC
                    for g in range(NG):
                        pss = []
                        for j in range(3):
                            ps_f = psf.tile([128, 512], f32, tag="f",
                                            name=f"ps_f{j}")
                            pss.append(ps_f)
                        for ti in range(3):
                            ta, tb = ti, ti + 3
                            for j in range(3):
                                n = 3 * g + j
                                nc.tensor.matmul(
                                    pss[j][0:64, :], sb_zt[:, ta, :],
                                    sb_w[:, HDLOC * ta + 512 * n:
                                         HDLOC * ta + 512 * (n + 1)],
                                    start=(ti == 0), stop=(ti == 2),
                                    tile_position=(0, 0))
                            for j in range(3):
                                n = 3 * g + j
                                nc.tensor.matmul(
                                    pss[j][64:128, :], sb_zt[:, tb, :],
                                    sb_w[:, HDLOC * tb + 512 * n:
                                         HDLOC * tb + 512 * (n + 1)],
                                    start=(ti == 0), stop=(ti == 2),
                                    tile_position=(0, 64))
                        for j in range(3):
                            n = 3 * g + j
                            dc, half = n // 2, n % 2
                            if tmpb[dc] is None:
                                tmpb[dc] = tbp.tile([128, 16, D], bf16,
                                                    tag="tmpb", name=f"tb{dc}")
                            # ACT: psum f32 -> sbuf bf16
                            nc.scalar.activation(
                                tmpb[dc][:, 8 * half:8 * (half + 1), :],
                                pss[j][:].rearrange("p (h d) -> p h d", d=D),
                                COPY)
                            if half == 1:
                                tmp2 = t2p.tile([128, 16, D], bf16,
                                                tag="tmp2", name="t2")
                                nc.vector.tensor_tensor(
                                    tmp2[:], tmpb[dc][:], sb_dxb[:, k], MUL)
                                nc.vector.tensor_reduce(
                                    sb_u[:, 16 * dc:16 * (dc + 1)],
                                    tmp2[:],
                                    axis=mybir.AxisListType.X, op=ADD)

                    # project z_k during the PE-idle tail (P only needed at
                    # the end)
                    project(k, sb_zt)

                    # fold kt halves: (64, 96)
                    sb_u2 = work.tile([B, HLOC], f32, tag="u2", name="u2")
                    nc.vector.tensor_copy(sb_u2[:], sb_u[64:128, :])
                    sb_uf = work.tile([B, HLOC], f32, tag="uf", name="uf")
                    nc.vector.tensor_tensor(
                        sb_uf[:], sb_u[0:64, :], sb_u2[:], ADD)

                    # transpose U -> (96, 64), update z slice, send + gather
                    ps_ut = pst.tile([128, 512], f32, tag="ut", name="ps_ut")
                    nc.tensor.transpose(ps_ut[0:HLOC, 0:B], sb_uf[:], sb_ident[:])
                    stt = nc.vector.scalar_tensor_tensor(
                        sb_zsl[:], ps_ut[0:HLOC, 0:B], float(dts[k]), sb_zsl[:],
                        op0=MUL, op1=ADD)
                    if USE_P2P:
                        # cast slice to bf16 into the send buffer; overwrite
                        # only after the previous broadcast fully sent
                        # (lsem wait patched onto a carrier nop post-sched)
                        cast = nc.vector.tensor_copy(sb_send[0:HLOC, :],
                                                     sb_zsl[:])
                        if k > 0:
                            wl = nc.vector.nop()
                            _add_dep_helper(wl.ins, stt.ins, sync=True,
                                            reason="anchor send gate in step")
                            _add_dep_helper(cast.ins, wl.ins, sync=True,
                                            reason="send buffer free gate")
                            sem_patches.append((wl, lsem, 16 * k))
                        nc.gpsimd.remote_dma_broadcast(
                            sb_stage[:, ts(rank_val, B)],
                            sb_send[:],
                            remote_sem=rsem, local_sem=lsem,
                            rdests=[(0, j) for j in range(NC)],
                        )
                        trig = nc.gpsimd.trigger_dma(count=None)
                        # carrier nop on the SP queue: gets the remote-
                        # arrival wait patched in post-scheduling
                        wr = nc.sync.nop()
                        _add_dep_helper(wr.ins, trig.ins, sync=True,
                                        reason="after p2p trigger")
                        sem_patches.append((wr, rsem, 16 * (k + 1)))
                        sb_zt = ztp.tile([128, KT, B], bf16, tag="zt",
                                         name="zt")
                        for (t, d0, d1, s, p0) in _tile_pieces():
                            mv = nc.sync.dma_start(
                                sb_zt[d0:d1, t, :],
                                sb_stage[p0:p0 + (d1 - d0),
                                         s * B:(s + 1) * B])
                            _add_dep_helper(mv.ins, wr.ins, sync=True,
                                            reason="p2p data arrival gate")
                    else:
                        sb_zslb = work.tile([HLOC, B], bf16, tag="zslb",
                                            name="zslb")
                        nc.vector.tensor_copy(sb_zslb[:], sb_zsl[:])
                        nc.sync.dma_start(zin_d[:], sb_zslb[:])
                        nc.gpsimd.collective_compute(
                            "AllGather", mybir.AluOpType.bypass,
                            replica_groups=[list(range(NC))],
                            ins=[zin_d[:]], outs=[zg_d.ap()],
                        )
                        # keep the PE HAM-warm through the collective window
                        # so the next burst runs at the warm clock
                        if NFILL and k < ns - 1:
                            ps_fill = pst.tile([128, 512], f32, tag="ut",
                                               name="ps_fill")
                            for fi in range(NFILL):
                                nc.tensor.matmul(
                                    ps_fill[0:B, :], sb_zslb[:],
                                    sb_w[0:HLOC, 512 * fi:512 * (fi + 1)],
                                    start=True, stop=True)
                        sb_zt = ztp.tile([128, KT, B], bf16, tag="zt",
                                         name="zt")
                        nc.sync.dma_start(
                            sb_zt[:],
                            zg_d.ap().rearrange("(t p) b -> p t b", p=128))

                project(ns, sb_zt)

                # ---- final interp + output --------------------------------
                # rows 0..19 of P are ready before the last gather: run the
                # big interp matmuls early; only the K=1 accumulation of
                # P[20] waits for the final projection.
                nc.sync.dma_start(sb_p[0:NS, :], p_d[:])
                sb_p20 = work.tile([1, B * OLOC], f32, tag="p20", name="p20")
                nc.sync.dma_start(sb_p20[:], p20_d[:])
                out_lbo = out_d.ap().rearrange("b l o -> l b o")
                BCH = 512 // OLOC
                for c in range(B * OLOC // 512):
                    ps_o = psf.tile([128, 512], f32, tag="f", name="ps_o")
                    nc.tensor.matmul(ps_o[0:L, :], sb_jt[0:NS, :],
                                     sb_p[0:NS, 512 * c:512 * (c + 1)],
                                     start=True, stop=False)
                    nc.tensor.matmul(ps_o[0:L, :], sb_jt20[:],
                                     sb_p20[:, 512 * c:512 * (c + 1)],
                                     start=False, stop=True)
                    sb_o = work.tile([L, 512], f32, tag="outstage", name="sb_o")
                    # alternate psum->sbuf copies across ACT and DVE
                    if c % 2 == 0:
                        nc.scalar.activation(sb_o[:], ps_o[0:L, :], COPY)
                    else:
                        nc.vector.tensor_copy(sb_o[:], ps_o[0:L, :])
                    nc.sync.dma_start(
                        out_lbo[:, BCH * c:BCH * (c + 1), :],
                        sb_o[:].rearrange("l (b o) -> l b o", o=OLOC))

    # inject the cross-core data-arrival waits after scheduling (the tile
    # scheduler's single-core sim cannot model remote semaphore increments
    # and would deadlock on them)
    for inst, sem, val in sem_patches:
        inst.wait_op(sem, val, "sem-ge", check=False)

    nc.compile()
    return nc


def _prepare(inputs):
    import ml_dtypes

    traj = np.asarray(inputs["traj"], dtype=np.float32)
    W_lin = np.asarray(inputs["W_lin"], dtype=np.float32)
    b_lin = np.asarray(inputs["b_lin"], dtype=np.float32)
    W_out = np.asarray(inputs["W_out"], dtype=np.float32)
    b_out = np.asarray(inputs["b_out"], dtype=np.float32)
    W_z0 = np.asarray(inputs["W_z0"], dtype=np.float32)
    b_z0 = np.asarray(inputs["b_z0"], dtype=np.float32)

    dts, mcoef, wv, JT = _host_constants()
    has_blin = bool(np.any(b_lin))
    has_bout = bool(np.any(b_out))
    if has_blin:
        raise NotImplementedError("b_lin != 0 not supported in fast path")

    key = (has_blin, has_bout)
    if key not in _prog_cache:
        _prog_cache[key] = _build_program(dts, has_bout)
    nc = _prog_cache[key]

    # host-side setup math (tiny)
    m1 = np.einsum('bld,l->bd', traj, wv).astype(np.float32)       # (B, D)
    base = (traj[:, 1, :] - traj[:, 0, :]).astype(np.float32)      # (B, D)
    dx = base[:, None, :] + mcoef[None, :, None] * m1[:, None, :]  # (B, NS, D)
    dx_dup = np.concatenate([dx, dx], axis=0)                      # (128, NS, D)
    dx_dup = np.ascontiguousarray(
        dx_dup.transpose(0, 1, 2).reshape(128, NS * D)).astype(np.float32)
    z0 = (traj[:, 0, :] @ W_z0.T + b_z0).astype(np.float32)        # (B, H)
    z0t = np.ascontiguousarray(z0.T)                               # (H, B)
    z0t_bf = z0t.astype(ml_dtypes.bfloat16)

    ident = np.eye(B, dtype=np.float32)
    WT_bf = np.ascontiguousarray(W_lin.T).astype(ml_dtypes.bfloat16)
    WO_bf = np.ascontiguousarray(W_out.T).astype(ml_dtypes.bfloat16)

    in_maps = []
    for i in range(NC):
        osl = slice(OLOC * i, OLOC * (i + 1))
        m = dict(
            wt_loc=np.ascontiguousarray(
                WT_bf[:, HLOC * D * i:HLOC * D * (i + 1)]),
            z0t=z0t_bf,
            z0l=np.ascontiguousarray(z0t[HLOC * i:HLOC * (i + 1), :]),
            dxdup=dx_dup,
            wo_loc=np.ascontiguousarray(WO_bf[:, osl]),
            jt=JT,
            ident=ident,
            rankin=np.array([[i]], np.int32),
        )
        if has_bout:
            m["bout_loc"] = np.ascontiguousarray(b_out[None, osl])
        in_maps.append(m)

    return nc, in_maps


def traced_run_args(inputs):
    """Build (nc, in_maps) exactly as kernel() would — for profiling."""
    return _prepare(inputs)


def kernel(**inputs):
    from concourse.bass_utils import run_bass_kernel_spmd

    nc, in_maps = _prepare(inputs)
    res = run_bass_kernel_spmd(nc, in_maps, core_ids=list(range(NC)))
    return np.concatenate([r["out"] for r in res.results], axis=2)


# revision 62
# speedup vs baseline: 1.0349x; 1.0004x over previous
"""Neural CDE encoder kernel for 8 Trainium2 NeuronCores.

Math (from the reference):
  - Natural cubic spline on unit-spaced knots; Euler times t_k = 0.05*k for
    k=0..19 all lie in interval [0,1), so only interval-0 coefficients matter:
        dX(t) = (y1 - y0) + M1 * (t^2/2 - 1/6)
    with M1 = <w, y> for a constant weight vector w over L. dX_k and z0 are
    precomputed on the host (tiny) and shipped as inputs.
  - Euler: z_{k+1} = z_k + dt * einsum('bhd,bd->bh', reshape(z W^T), dX_k)
  - Output: project grid z's with W_out, then linearly interpolate via a
    constant (L x 21) matrix.

Sharding: tensor-parallel over H (96 rows of H per core; 6144 rows of W_lin).
W^T shard SBUF-resident in bf16 (768, 6144). Per step:
  - main matmul in bf16 with 2-way col-tiling (tile_position (0,0)/(0,64)):
    kt 0-2 accumulate into psum partitions 0-63, kt 3-5 into 64-127; n-chunks
    processed in groups of 3 so each stationary load covers 3 matmuls.
  - ACT engine copies psum chunks to SBUF bf16 (pipelines behind the matmul
    burst); DVE does the dX multiply and d-reduce on bf16 double-chunks
    (1024-wide ops) against pre-broadcast bf16 dX tiles built once at
    startup; kt halves folded once per step.
  - z state kept in f32; z slice cast to bf16 and AllGathered in bf16 each
    step (half the wire bytes); gathered bf16 loads directly as the next
    stationary tiles (no post-gather cast).
  - a tiny warmup AllGather at startup absorbs the ~12us cold-collective
    cost; filler matmuls through each collective window keep the PE's HAM
    clock-gate from re-throttling the next burst.
W_out sharded over O (32 cols/core, bf16); per-step projections are emitted
after the main burst (they only feed the final output), staged to DRAM P;
the final interp matmul is split so the K=20 part runs during the last
gather and only a K=1 accumulation of P[20] trails it.
(USE_P2P: an SBUF-to-SBUF remote_dma_broadcast gather exists but is disabled
— the axon runtime wedges on SWDGE remote DMA.)
"""

import numpy as np

B, L, D, H, O = 64, 128, 64, 768, 256
NS = 20            # Euler steps
NC = 8             # cores
HLOC = H // NC     # 96
HDLOC = HLOC * D   # 6144
OLOC = O // NC     # 32
KT = H // 128      # 6 contraction tiles
NT = HDLOC // 512  # 12 moving chunks per step
NG = 4             # n-chunk groups of 3
NDC = NT // 2      # 6 double-chunks for DVE
USE_P2P = False    # hand-rolled remote-DMA gather vs collective AllGather
NFILL = 36         # HAM-warming filler matmuls per collective window


def _tile_pieces():
    """Static (tile, dst_p0, dst_p1, slot, src_p0) pieces mapping the
    8x96-row gathered slices onto 6x128-row stationary tiles."""
    pieces = []
    for t in range(KT):
        covered = 0
        while covered < 128:
            r = 128 * t + covered
            s, off = divmod(r, HLOC)
            ln = min(HLOC - off, 128 - covered)
            pieces.append((t, covered, covered + ln, s, off))
            covered += ln
    return pieces

_prog_cache = {}


def _host_constants():
    grid = (np.arange(NS + 1, dtype=np.float32) * np.float32(0.05)).astype(np.float32)
    grid[-1] = np.float32(1.0)
    dts = (grid[1:] - grid[:-1]).astype(np.float32)
    tk = grid[:-1].astype(np.float64)
    mcoef = (tk * tk / 2.0 - 1.0 / 6.0).astype(np.float32)

    # w over L such that M1 = <w, y>
    n = L - 2
    A = 4.0 * np.eye(n) + np.eye(n, k=1) + np.eye(n, k=-1)
    r0 = np.linalg.solve(A, np.eye(n)[:, 0])
    w = np.zeros(L, dtype=np.float64)
    w[0:n] += 6.0 * r0
    w[1:n + 1] += -12.0 * r0
    w[2:n + 2] += 6.0 * r0

    # Interp matrix J (L, NS+1)
    ts = np.linspace(0.0, 1.0, L, dtype=np.float32)
    j = np.clip(np.searchsorted(grid, ts, side="right") - 1, 0, NS - 1)
    wl = ((ts - grid[j]) / (grid[j + 1] - grid[j])).astype(np.float32)
    J = np.zeros((L, NS + 1), dtype=np.float32)
    J[np.arange(L), j] += 1.0 - wl
    J[np.arange(L), j + 1] += wl
    return dts, mcoef, w.astype(np.float32), J.T.copy()  # JT (21, 128)


def _build_program(dts, has_bout, ns=NS):
    import concourse.bacc as bacc
    import concourse.mybir as mybir
    import concourse.tile as tile
    from concourse.bass import ts, _add_dep_helper

    f32 = mybir.dt.float32
    bf16 = mybir.dt.bfloat16
    ADD = mybir.AluOpType.add
    MUL = mybir.AluOpType.mult
    COPY = mybir.ActivationFunctionType.Copy

    nc = bacc.Bacc("TRN2", target_bir_lowering=False, debug=False, num_devices=NC)

    # ---- I/O -------------------------------------------------------------
    wt_d = nc.dram_tensor("wt_loc", [H, HDLOC], bf16, kind="ExternalInput")
    z0t_d = nc.dram_tensor("z0t", [H, B], bf16, kind="ExternalInput")
    z0l_d = nc.dram_tensor("z0l", [HLOC, B], f32, kind="ExternalInput")
    rank_d = nc.dram_tensor("rankin", [1, 1], mybir.dt.int32, kind="ExternalInput")
    dx_d = nc.dram_tensor("dxdup", [128, NS * D], f32, kind="ExternalInput")
    wo_d = nc.dram_tensor("wo_loc", [H, OLOC], bf16, kind="ExternalInput")
    jt_d = nc.dram_tensor("jt", [NS + 1, L], f32, kind="ExternalInput")
    id_d = nc.dram_tensor("ident", [B, B], f32, kind="ExternalInput")
    if has_bout:
        bout_d = nc.dram_tensor("bout_loc", [1, OLOC], f32, kind="ExternalInput")
    out_d = nc.dram_tensor("out", [B, L, OLOC], f32, kind="ExternalOutput")

    if USE_P2P:
        rsem = nc.alloc_semaphore("rsem")
        lsem = nc.alloc_semaphore("lsem")
    zg_d = nc.dram_tensor("zgather", [H, B], bf16, kind="Internal",
                          addr_space="Shared")
    sem_patches = []  # (BassInstruction, sem, threshold) applied post-scheduling
    # tiny warmup collective buffers (acts as a startup barrier)
    wu_in_d = nc.dram_tensor("wuin", [1, 64], f32, kind="Internal")
    wu_out_d = nc.dram_tensor("wuout", [NC, 64], f32, kind="Internal",
                              addr_space="Shared")

    with tile.TileContext(nc) as tc:
        with (
            tc.tile_pool(name="pers", bufs=1) as pers,
            tc.tile_pool(name="ztpool", bufs=2) as ztp,
            tc.tile_pool(name="dram", bufs=1, space="DRAM") as dram,
        ):
            # warmup collective, independent of everything else: scheduler
            # runs it during the initial weight DMAs.
            sb_wu = pers.tile([1, 64], f32, tag="wu")
            nc.vector.memset(sb_wu[:], 0.0)
            nc.sync.dma_start(wu_in_d.ap(), sb_wu[:])
            nc.gpsimd.collective_compute(
                "AllGather", mybir.AluOpType.bypass,
                replica_groups=[list(range(NC))],
                ins=[wu_in_d.ap()], outs=[wu_out_d.ap()],
            )

            # z0 stationary first: step-0's burst only needs this + W, and
            # can then fully overlap the startup barrier
            sb_zt = ztp.tile([128, KT, B], bf16, tag="zt", name="zt0")
            nc.sync.dma_start(
                sb_zt[:],
                z0t_d.ap().rearrange("(t p) b -> p t b", p=128))

            # persistent tiles
            sb_wot = pers.tile([128, KT * OLOC], bf16, tag="wot")
            for t in range(KT):
                nc.sync.dma_start(sb_wot[:, OLOC * t:OLOC * (t + 1)],
                                  wo_d[128 * t:128 * (t + 1), :])
            sb_ident = pers.tile([B, B], f32, tag="ident")
            nc.sync.dma_start(sb_ident[:], id_d[:])
            sb_ident2 = pers.tile([128, B], f32, tag="ident2")
            nc.sync.dma_start(sb_ident2[0:B, :], id_d[:])
            nc.sync.dma_start(sb_ident2[B:128, :], id_d[:])
            sb_jt = pers.tile([NS + 1, L], f32, tag="jt")
            nc.sync.dma_start(sb_jt[:], jt_d[:])
            sb_jt20 = pers.tile([1, L], f32, tag="jt20")
            nc.sync.dma_start(sb_jt20[:], jt_d[NS:NS + 1, :])
            sb_dx = pers.tile([128, NS * D], f32, tag="dx")
            nc.sync.dma_start(sb_dx[:], dx_d[:])
            sb_zsl = pers.tile([HLOC, B], f32, tag="zsl")
            nc.sync.dma_start(sb_zsl[:], z0l_d[:])
            sb_p = pers.tile([NS + 1, B * OLOC], f32, tag="P")
            if USE_P2P:
                # P2P gather state
                sb_rank = pers.tile([1, 1], mybir.dt.int32, tag="rank")
                nc.sync.dma_start(sb_rank[:], rank_d[:])
                sb_send = pers.tile([128, B], bf16, tag="send")
                nc.vector.memset(sb_send[:], 0.0)
                sb_stage = pers.tile([128, NC * B], bf16, tag="stage")
                nc.vector.memset(sb_stage[:], 0.0)
                rank_val = nc.gpsimd.value_load(sb_rank[0:1, 0:1], min_val=0,
                                                max_val=NC - 1)
            if has_bout:
                sb_bout = pers.tile([1, OLOC], f32, tag="bout")
                nc.sync.dma_start(sb_bout[:], bout_d[:])
                sb_ones = pers.tile([1, B], f32, tag="ones")
                nc.vector.memset(sb_ones[:], 1.0)

            # dX broadcast tiles for all steps, built once on ACT during the
            # startup window: [128, NS, 16, D] bf16
            sb_dxb = pers.tile([128, NS, 16, D], bf16, tag="dxb")
            for k in range(ns):
                nc.scalar.activation(
                    sb_dxb[:, k],
                    sb_dx[:, D * k:D * (k + 1)][:, None, :]
                    .to_broadcast((128, 16, D)),
                    COPY)

            # W^T shard, bf16, 6 kt-tiles side by side (128, 6*6144)
            sb_w = pers.tile([128, KT * HDLOC], bf16, tag="W")
            for t in range(KT):
                for cc in range(4):
                    nc.sync.dma_start(
                        sb_w[:, HDLOC * t + 1536 * cc:HDLOC * t + 1536 * (cc + 1)],
                        wt_d[128 * t:128 * (t + 1),
                             1536 * cc:1536 * (cc + 1)])

            zin_d = dram.tile([HLOC, B], bf16)
            p_d = dram.tile([NS, B * OLOC], f32)
            p20_d = dram.tile([1, B * OLOC], f32)

            # ---- main loop -------------------------------------------------
            with (
                tc.tile_pool(name="work", bufs=3) as work,
                tc.tile_pool(name="tbp", bufs=3) as tbp,
                tc.tile_pool(name="t2p", bufs=2) as t2p,
                tc.tile_pool(name="upool", bufs=2) as upool,
                tc.tile_pool(name="psf", bufs=6, space="PSUM") as psf,
                tc.tile_pool(name="pst", bufs=1, space="PSUM") as pst,
                tc.tile_pool(name="psp", bufs=1, space="PSUM") as psp,
            ):
                def project(k, zt):
                    ps_p = psp.tile([128, 512], f32, tag="pp", name="ps_p")
                    if has_bout:
                        nc.tensor.matmul(ps_p[0:B, 0:OLOC], sb_ones[:], sb_bout[:],
                                         start=True, stop=False)
                    for t in range(KT):
                        nc.tensor.matmul(
                            ps_p[0:B, 0:OLOC], zt[:, t, :],
                            sb_wot[:, OLOC * t:OLOC * (t + 1)],
                            start=(t == 0 and not has_bout), stop=(t == KT - 1))
                    sb_pst = work.tile([B, OLOC], f32, tag="pstage", name="pst")
                    nc.scalar.activation(sb_pst[:], ps_p[0:B, 0:OLOC], COPY)
                    tgt = p20_d[0, :] if k == NS else p_d[k, :]
                    nc.sync.dma_start(
                        tgt.rearrange("(b o) -> b o", b=B), sb_pst[:])

                for k in range(ns):
                    sb_u = upool.tile([128, HLOC], f32, tag="U", name="u")
                    tmpb = [None] * NDC
                    for g in range(NG):
                        pss = []
                        for j in range(3):
                            ps_f = psf.tile([128, 512], f32, tag="f",
                                            name=f"ps_f{j}")
                            pss.append(ps_f)
                        for ti in range(3):
                            ta, tb = ti, ti + 3
                            for j in range(3):
                                n = 3 * g + j
                                nc.tensor.matmul(
                                    pss[j][0:64, :], sb_zt[:, ta, :],
                                    sb_w[:, HDLOC * ta + 512 * n:
                                         HDLOC * ta + 512 * (n + 1)],
                                    start=(ti == 0), stop=(ti == 2),
                                    tile_position=(0, 0))
                            for j in range(3):
                                n = 3 * g + j
                                nc.tensor.matmul(
                                    pss[j][64:128, :], sb_zt[:, tb, :],
                                    sb_w[:, HDLOC * tb + 512 * n:
                                         HDLOC * tb + 512 * (n + 1)],
                                    start=(ti == 0), stop=(ti == 2),
                                    tile_position=(0, 64))
                        for j in range(3):
                            n = 3 * g + j
                            dc, half = n // 2, n % 2
                            if tmpb[dc] is None:
                                tmpb[dc] = tbp.tile([128, 16, D], bf16,
                                                    tag="tmpb", name=f"tb{dc}")
                            # ACT: psum f32 -> sbuf bf16
                            nc.scalar.activation(
                                tmpb[dc][:, 8 * half:8 * (half + 1), :],
                                pss[j][:].rearrange("p (h d) -> p h d", d=D),
                                COPY)
                            if half == 1:
                                tmp2 = t2p.tile([128, 16, D], bf16,
                                                tag="tmp2", name="t2")
                                nc.vector.tensor_tensor(
                                    tmp2[:], tmpb[dc][:], sb_dxb[:, k], MUL)
                                nc.vector.tensor_reduce(
                                    sb_u[:, 16 * dc:16 * (dc + 1)],
                                    tmp2[:],
                                    axis=mybir.AxisListType.X, op=ADD)

                    # project z_k during the PE-idle tail (P only needed at
                    # the end)
                    project(k, sb_zt)

                    # fold kt halves: (64, 96)
                    sb_u2 = work.tile([B, HLOC], f32, tag="u2", name="u2")
                    nc.vector.tensor_copy(sb_u2[:], sb_u[64:128, :])
                    sb_uf = work.tile([B, HLOC], f32, tag="uf", name="uf")
                    nc.vector.tensor_tensor(
                        sb_uf[:], sb_u[0:64, :], sb_u2[:], ADD)

                    # transpose U -> (96, 64), update z slice, send + gather
                    ps_ut = pst.tile([128, 512], f32, tag="ut", name="ps_ut")
                    nc.tensor.transpose(ps_ut[0:HLOC, 0:B], sb_uf[:], sb_ident[:])
                    stt = nc.vector.scalar_tensor_tensor(
                        sb_zsl[:], ps_ut[0:HLOC, 0:B], float(dts[k]), sb_zsl[:],
                        op0=MUL, op1=ADD)
                    if USE_P2P:
                        # cast slice to bf16 into the send buffer; overwrite
                        # only after the previous broadcast fully sent
                        # (lsem wait patched onto a carrier nop post-sched)
                        cast = nc.vector.tensor_copy(sb_send[0:HLOC, :],
                                                     sb_zsl[:])
                        if k > 0:
                            wl = nc.vector.nop()
                            _add_dep_helper(wl.ins, stt.ins, sync=True,
                                            reason="anchor send gate in step")
                            _add_dep_helper(cast.ins, wl.ins, sync=True,
                                            reason="send buffer free gate")
                            sem_patches.append((wl, lsem, 16 * k))
                        nc.gpsimd.remote_dma_broadcast(
                            sb_stage[:, ts(rank_val, B)],
                            sb_send[:],
                            remote_sem=rsem, local_sem=lsem,
                            rdests=[(0, j) for j in range(NC)],
                        )
                        trig = nc.gpsimd.trigger_dma(count=None)
                        # carrier nop on the SP queue: gets the remote-
                        # arrival wait patched in post-scheduling
                        wr = nc.sync.nop()
                        _add_dep_helper(wr.ins, trig.ins, sync=True,
                                        reason="after p2p trigger")
                        sem_patches.append((wr, rsem, 16 * (k + 1)))
                        sb_zt = ztp.tile([128, KT, B], bf16, tag="zt",
                                         name="zt")
                        for (t, d0, d1, s, p0) in _tile_pieces():
                            mv = nc.sync.dma_start(
                                sb_zt[d0:d1, t, :],
                                sb_stage[p0:p0 + (d1 - d0),
                                         s * B:(s + 1) * B])
                            _add_dep_helper(mv.ins, wr.ins, sync=True,
                                            reason="p2p data arrival gate")
                    else:
                        sb_zslb = work.tile([HLOC, B], bf16, tag="zslb",
                                            name="zslb")
                        nc.vector.tensor_copy(sb_zslb[:], sb_zsl[:])
                        nc.sync.dma_start(zin_d[:], sb_zslb[:])
                        nc.gpsimd.collective_compute(
                            "AllGather", mybir.AluOpType.bypass,
                            replica_groups=[list(range(NC))],
                            ins=[zin_d[:]], outs=[zg_d.ap()],
                        )
                        # keep the PE HAM-warm through the collective window
                        # so the next burst runs at the warm clock
                        if NFILL and k < ns - 1:
                            ps_fill = pst.tile([128, 512], f32, tag="ut",
                                               name="ps_fill")
                            for fi in range(NFILL):
                                nc.tensor.matmul(
                                    ps_fill[0:B, :], sb_zslb[:],
                                    sb_w[0:HLOC, 512 * fi:512 * (fi + 1)],
                                    start=True, stop=True)
                        sb_zt = ztp.tile([128, KT, B], bf16, tag="zt",
                                         name="zt")
                        nc.sync.dma_start(
                            sb_zt[:],
                            zg_d.ap().rearrange("(t p) b -> p t b", p=128))

                project(ns, sb_zt)

                # ---- final interp + output --------------------------------
                # rows 0..19 of P are ready before the last gather: run the
                # big interp matmuls early; only the K=1 accumulation of
                # P[20] waits for the final projection.
                nc.sync.dma_start(sb_p[0:NS, :], p_d[:])
                sb_p20 = work.tile([1, B * OLOC], f32, tag="p20", name="p20")
                nc.sync.dma_start(sb_p20[:], p20_d[:])
                out_lbo = out_d.ap().rearrange("b l o -> l b o")
                BCH = 512 // OLOC
                # J^T row 20 only contributes to output rows >= 121 (the
                # last grid interval): rows 0:96 are final after the K=20
                # matmul and can be copied/DMA'd before P[20] exists (the
                # K=1 matmul covers rows 96:128, adding zeros for 96:121).
                L0J = 96
                for c in range(B * OLOC // 512):
                    ps_o = psf.tile([128, 512], f32, tag="f", name="ps_o")
                    nc.tensor.matmul(ps_o[0:L, :], sb_jt[0:NS, :],
                                     sb_p[0:NS, 512 * c:512 * (c + 1)],
                                     start=True, stop=True)
                    sb_o = work.tile([L, 512], f32, tag="outstage", name="sb_o")
                    # alternate psum->sbuf copies across ACT and DVE
                    if c % 2 == 0:
                        nc.scalar.activation(sb_o[0:L0J, :], ps_o[0:L0J, :],
                                             COPY)
                    else:
                        nc.vector.tensor_copy(sb_o[0:L0J, :], ps_o[0:L0J, :])
                    nc.sync.dma_start(
                        out_lbo[0:L0J, BCH * c:BCH * (c + 1), :],
                        sb_o[0:L0J, :].rearrange("l (b o) -> l b o", o=OLOC))
                    nc.tensor.matmul(ps_o[L0J:L, :], sb_jt20[:, L0J:L],
                                     sb_p20[:, 512 * c:512 * (c + 1)],
                                     start=False, stop=True,
                                     tile_position=(0, L0J))
                    if c % 2 == 0:
                        nc.scalar.activation(sb_o[L0J:L, :], ps_o[L0J:L, :],
                                             COPY)
                    else:
                        nc.vector.tensor_copy(sb_o[L0J:L, :], ps_o[L0J:L, :])
                    nc.sync.dma_start(
                        out_lbo[L0J:L, BCH * c:BCH * (c + 1), :],
                        sb_o[L0J:L, :].rearrange("l (b o) -> l b o", o=OLOC))

    # inject the cross-core data-arrival waits after scheduling (the tile
    # scheduler's single-core sim cannot model remote semaphore increments
    # and would deadlock on them)
    for inst, sem, val in sem_patches:
        inst.wait_op(sem, val, "sem-ge", check=False)

    nc.compile()
    return nc


def _prepare(inputs):
    import ml_dtypes

    traj = np.asarray(inputs["traj"], dtype=np.float32)
    W_lin = np.asarray(inputs["W_lin"], dtype=np.float32)
    b_lin = np.asarray(inputs["b_lin"], dtype=np.float32)
    W_out = np.asarray(inputs["W_out"], dtype=np.float32)
    b_out = np.asarray(inputs["b_out"], dtype=np.float32)
    W_z0 = np.asarray(inputs["W_z0"], dtype=np.float32)
    b_z0 = np.asarray(inputs["b_z0"], dtype=np.float32)

    dts, mcoef, wv, JT = _host_constants()
    has_blin = bool(np.any(b_lin))
    has_bout = bool(np.any(b_out))
    if has_blin:
        raise NotImplementedError("b_lin != 0 not supported in fast path")

    key = (has_blin, has_bout)
    if key not in _prog_cache:
        _prog_cache[key] = _build_program(dts, has_bout)
    nc = _prog_cache[key]

    # host-side setup math (tiny)
    m1 = np.einsum('bld,l->bd', traj, wv).astype(np.float32)       # (B, D)
    base = (traj[:, 1, :] - traj[:, 0, :]).astype(np.float32)      # (B, D)
    dx = base[:, None, :] + mcoef[None, :, None] * m1[:, None, :]  # (B, NS, D)
    dx_dup = np.concatenate([dx, dx], axis=0)                      # (128, NS, D)
    dx_dup = np.ascontiguousarray(
        dx_dup.transpose(0, 1, 2).reshape(128, NS * D)).astype(np.float32)
    z0 = (traj[:, 0, :] @ W_z0.T + b_z0).astype(np.float32)        # (B, H)
    z0t = np.ascontiguousarray(z0.T)                               # (H, B)
    z0t_bf = z0t.astype(ml_dtypes.bfloat16)

    ident = np.eye(B, dtype=np.float32)
    WT_bf = np.ascontiguousarray(W_lin.T).astype(ml_dtypes.bfloat16)
    WO_bf = np.ascontiguousarray(W_out.T).astype(ml_dtypes.bfloat16)

    in_maps = []
    for i in range(NC):
        osl = slice(OLOC * i, OLOC * (i + 1))
        m = dict(
            wt_loc=np.ascontiguousarray(
                WT_bf[:, HLOC * D * i:HLOC * D * (i + 1)]),
            z0t=z0t_bf,
            z0l=np.ascontiguousarray(z0t[HLOC * i:HLOC * (i + 1), :]),
            dxdup=dx_dup,
            wo_loc=np.ascontiguousarray(WO_bf[:, osl]),
            jt=JT,
            ident=ident,
            rankin=np.array([[i]], np.int32),
        )
        if has_bout:
            m["bout_loc"] = np.ascontiguousarray(b_out[None, osl])
        in_maps.append(m)

    return nc, in_maps


def traced_run_args(inputs):
    """Build (nc, in_maps) exactly as kernel() would — for profiling."""
    return _prepare(inputs)


def kernel(**inputs):
    from concourse.bass_utils import run_bass_kernel_spmd

    nc, in_maps = _prepare(inputs)
    res = run_bass_kernel_spmd(nc, in_maps, core_ids=list(range(NC)))
    return np.concatenate([r["out"] for r in res.results], axis=2)
